# revision 26
# baseline (speedup 1.0000x reference)
"""Trainium2 Bass kernel for a GNN message-passing layer (8-core SPMD), v2.

Math (reference):
    z   = [x[row], x[col], edge_attr] @ W1 + b1        # [E, 258] @ [258, 128]
    m   = relu(LN(z, g1, be1))
    m   = relu(m @ W2 + b2)
    aggr= segment_sum(m, row, N)
    u   = relu(LN([x, aggr] @ Wu + bu, gu, beu))
    out = x + u

Design (v2 — no device gathers at all):
  The host shards edges by source-row range (nloc rows per core), sorts each
  shard by 128-row window, and pre-gathers x[row], x[col] per edge into
  transposed bf16 tile streams.  The device streams those sequentially
  (HWDGE DMA) and computes, per 128-edge tile group (4 tiles = N=512):

    zT [D, e] = W1a^T @ xrowT + W1b^T @ xcolT + [c0;c1;1;b1]^T @ [a0;a1;-mu;v]
      (mean subtracted inside the PSUM accumulation: -mu is a host-computed
       per-edge scalar = -(h.w1sum + sum b1)/D, a linear functional of the
       inputs; v is the valid mask so pad edges stay exactly zero)
    m1u = relu(g1 * zT)            # ACT batched, rstd deferred (exact: rstd>0)
    m2T = relu(W2s^T @ m1u)        # W2s = W2 * sqrt(D), rstd deferred again
    zsq = Square(zT)               # ACT batched -> DMA-transpose -> DVE
    sumsq[e] = reduce(zsq)         #   row-sum; var = sumsq/D (mean folded)
    rr = 1/sqrt(sumsq + D*eps)     # per window: ACT Sqrt + DVE reciprocal
    oh = (iota == sidx) * rr       # one DVE tensor_scalar, bf16
    aggT [D, slot] += m2^T @ oh    # PE scatter; m2 via DMA-transpose of m2T

  Update MLP per 128-node window: up = aggT^T@Wu_a + x@Wu_x + bu, LN+relu on
  ACT/DVE, residual add, DMA out.  No collectives, no GPSIMD, no dma_gather.

  One NEFF for all 8 cores: per-window tile counts are maxed across cores;
  pad edges are all-zero with sidx=-1 (one-hot column is all zero).
"""

import math
import os
import numpy as np
import ml_dtypes

BF16 = ml_dtypes.bfloat16

D = 128
N_NODES = 50000
N_EDGES = 800000
N_CORES = 8
EPS = 1e-5
P = 128
G = 4  # tiles per matmul group (N = G*128 = 512 <= one PSUM bank of fp32)


class Cfg:
    def __init__(self, n_nodes, n_edges, n_cores):
        self.n_nodes = n_nodes
        self.n_edges = n_edges
        self.n_cores = n_cores
        assert n_nodes % n_cores == 0
        self.nloc = n_nodes // n_cores
        self.n_win = math.ceil(self.nloc / P)
        self.nloc_pad = self.n_win * P
        self.nt = None        # [n_win] tiles per window (maxed over cores)
        self.t_total = None
        self.e_pad = None
        self.max_ntw = None
        self.wstart = None    # [n_win] first tile index of each window
        self.perms = None     # per-core [n_win, 128] node permutation
        self.triv = None


# ---------------------------------------------------------------- host prep

def _pack_windows(deg, n_win, cap):
    """First/best-fit-decreasing: assign nodes to n_win bins of <=128 nodes,
    targeting <=cap edges per bin.  Returns win_of[node], bin edge counts."""
    nloc = len(deg)
    order = np.argsort(-deg, kind="stable")
    bins_e = np.zeros(n_win, dtype=np.int64)
    bins_n = np.zeros(n_win, dtype=np.int64)
    win_of = np.full(nloc, -1, dtype=np.int64)
    for n in order:
        d = int(deg[n])
        ok = (bins_e + d <= cap) & (bins_n < P)
        if ok.any():
            cand = np.where(ok)[0]
            b = cand[np.argmax(bins_e[cand])]          # fullest that fits
        else:
            cand = np.where(bins_n < P)[0]
            b = cand[np.argmin(bins_e[cand])]          # least-loaded spill
        win_of[n] = b
        bins_e[b] += d
        bins_n[b] += 1
    return win_of, bins_e


def preprocess(cfg, x, edge_index, edge_attr, w1sum, b1sum):
    """Shard edges, bin-pack nodes into 128-node windows with balanced edge
    counts, pre-gather x rows, build per-core bf16 streams."""
    rows = np.asarray(edge_index[0], dtype=np.int64)
    cols = np.asarray(edge_index[1], dtype=np.int64)
    attr = np.asarray(edge_attr, dtype=np.float32)

    # per-node linear functionals for the LN mean
    xw1a = x @ w1sum[0:D]          # [N]
    xw1b = x @ w1sum[D:2 * D]      # [N]

    order = np.argsort(rows, kind="stable")
    rs = rows[order]
    bounds = np.searchsorted(rs, np.arange(cfg.n_cores + 1) * cfg.nloc)

    per_core = []
    counts = np.zeros((cfg.n_cores, cfg.n_win), dtype=np.int64)
    perms = []
    for k in range(cfg.n_cores):
        sel = order[bounds[k]:bounds[k + 1]]
        row = rows[sel]
        row_l = (row - k * cfg.nloc).astype(np.int64)
        col = cols[sel]
        at = attr[sel]

        deg = np.bincount(row_l, minlength=cfg.nloc)
        cap = P * max(1, int(math.ceil(len(row_l) / (cfg.n_win * P))))
        win_of, bins_e = _pack_windows(deg, cfg.n_win, cap)
        rank = np.argsort(-bins_e, kind="stable")     # big windows first
        rankpos = np.empty(cfg.n_win, dtype=np.int64)
        rankpos[rank] = np.arange(cfg.n_win)
        wpos_of = rankpos[win_of]                     # node -> window index
        # slots within each window, and the node permutation
        o_n = np.argsort(wpos_of, kind="stable")
        slot_of = np.empty(cfg.nloc, dtype=np.int64)
        perm = np.full((cfg.n_win, P), -1, dtype=np.int64)
        wcnt = np.bincount(wpos_of, minlength=cfg.n_win)
        off = 0
        for w in range(cfg.n_win):
            nn = int(wcnt[w])
            nodes = o_n[off:off + nn]
            slot_of[nodes] = np.arange(nn)
            perm[w, :nn] = nodes
            off += nn
        perms.append(perm)

        key = wpos_of[row_l]
        o2 = np.argsort(key, kind="stable")
        row_l, col, at, row = row_l[o2], col[o2], at[o2], row[o2]
        counts[k] = np.bincount(key[o2], minlength=cfg.n_win)
        per_core.append((slot_of[row_l], col, at, row))

    cfg.perms = perms
    nt = np.maximum(np.ceil(counts.max(axis=0) / P).astype(np.int64), 1)
    cfg.nt = nt
    cfg.t_total = int(nt.sum())
    cfg.e_pad = cfg.t_total * P
    cfg.max_ntw = int(nt.max())
    cfg.wstart = np.concatenate([[0], np.cumsum(nt)[:-1]]).astype(np.int64)

    x_bf = x.astype(BF16)
    core_arrays = []
    for k in range(cfg.n_cores):
        slot, col, at, row = per_core[k]
        # destination position of each (sorted) edge in the padded stream
        pos = np.empty(len(slot), dtype=np.int64)
        src = 0
        for w in range(cfg.n_win):
            c = int(counts[k, w])
            pos[src:src + c] = cfg.wstart[w] * P + np.arange(c)
            src += c

        xrow = np.zeros((cfg.e_pad, D), dtype=BF16)
        xcol = np.zeros((cfg.e_pad, D), dtype=BF16)
        combo = np.zeros((4, cfg.e_pad), dtype=np.float32)
        sidx = np.full(cfg.e_pad, -1.0, dtype=np.float32)

        xrow[pos] = x_bf[row]
        xcol[pos] = x_bf[col]
        combo[0, pos] = at[:, 0]
        combo[1, pos] = at[:, 1]
        negmu = -(xw1a[row] + xw1b[col]
                  + at[:, 0] * w1sum[2 * D] + at[:, 1] * w1sum[2 * D + 1]
                  + b1sum) / D
        combo[2, pos] = negmu
        combo[3, pos] = 1.0
        sidx[pos] = slot.astype(np.float32)

        # per-window [xrowT_w | xcolT_w] stream: one DMA/window.  The
        # scatter one-hot is built on the (otherwise idle) GPSIMD engine
        # from the resident sidxT table; rstd is applied on the m2 side.
        xrowT = xrow.T
        xcolT = xcol.T
        xw = np.zeros((P, 2 * cfg.e_pad), dtype=BF16)
        for w in range(cfg.n_win):
            W = int(cfg.nt[w]) * P
            t0c = int(cfg.wstart[w]) * P
            b = 2 * t0c
            xw[:, b:b + W] = xrowT[:, t0c:t0c + W]
            xw[:, b + W:b + 2 * W] = xcolT[:, t0c:t0c + W]
        core_arrays.append(dict(
            xw=xw,
            combo4=combo.astype(BF16),
            sidxT=np.ascontiguousarray(
                sidx.reshape(cfg.t_total, P).T),
        ))
    return core_arrays


# ---------------------------------------------------------------- device IR

def build(nc, tc, cfg, io):
    from concourse import mybir
    from contextlib import ExitStack
    import concourse.bass as bass

    f32 = mybir.dt.float32
    bf16 = mybir.dt.bfloat16
    AF = mybir.ActivationFunctionType
    OP = mybir.AluOpType
    AX = mybir.AxisListType
    triv = cfg.triv

    ctx = ExitStack()
    sing = ctx.enter_context(tc.tile_pool(name="sing", bufs=1))
    gin = ctx.enter_context(tc.tile_pool(name="gin", bufs=3))
    gmid = ctx.enter_context(tc.tile_pool(name="gmid", bufs=2))
    tsm = ctx.enter_context(tc.tile_pool(name="tsm", bufs=3))
    wbuf = ctx.enter_context(tc.tile_pool(name="wbuf", bufs=2))
    stat = ctx.enter_context(tc.tile_pool(name="stat", bufs=2))
    psA = ctx.enter_context(tc.tile_pool(name="psA", bufs=3, space="PSUM"))
    psC = ctx.enter_context(tc.tile_pool(name="psC", bufs=2, space="PSUM"))
    psT = ctx.enter_context(tc.tile_pool(name="psT", bufs=3, space="PSUM"))

    # ---------------- resident weights / constants
    def load(name, shape, dt):
        t = sing.tile(shape, dt, name=f"{name}_sb")
        nc.sync.dma_start(out=t[:], in_=io[name][:])
        return t

    w1a = load("W1a", [P, D], bf16)
    w1b = load("W1b", [P, D], bf16)
    cmb = load("combo_lhs", [4, D], bf16)
    w2s = load("W2s", [P, D], bf16)
    wux = load("Wu_x", [P, D], bf16)
    wua = load("Wu_a", [P, D], bf16)
    bu_row = load("bu_row", [1, D], bf16)
    sidxT = load("sidxT", [P, cfg.t_total], f32)
    xupT = load("xupT", [P, cfg.nloc_pad], bf16)

    def bcast_row(name, dt):
        t = sing.tile([P, D], dt, name=f"{name}_b")
        src = io[name]
        ap = bass.AP(tensor=src.tensor, offset=src.offset,
                     ap=[[0, P]] + list(src.ap))
        nc.sync.dma_start(out=t[:], in_=ap)
        return t

    iota_b = bcast_row("iota", bf16)
    g1_col = None if triv["g1"] else load("g1_col", [P, 1], f32)
    gu_b = None if triv["gu"] else bcast_row("gu", f32)
    beu_b = None if triv["beu"] else bcast_row("beu", f32)

    ones_row = sing.tile([1, P], bf16, name="ones_row")
    nc.vector.memset(ones_row[:], 1.0)
    ones_col = sing.tile([P, 1], bf16, name="ones_col")
    nc.vector.memset(ones_col[:], 1.0)
    epsD_t = sing.tile([P, 1], f32, name="epsD_t")
    nc.vector.memset(epsD_t[:], float(D * EPS))
    eps_t = sing.tile([P, 1], f32, name="eps_t")
    nc.vector.memset(eps_t[:], EPS)

    from concourse.masks import make_identity
    ident_bf = sing.tile([P, P], bf16, name="ident_bf")
    make_identity(nc, ident_bf[:])

    out_dram = io["out"]
    max_ntw = cfg.max_ntw

    # per-window state produced by the group phase, consumed by scatter
    def _emit_sumsq(wps, ent):
        gg, gsz, zsqT = ent
        for i in range(gsz):
            ti = gg + i
            nc.tensor.matmul(wps[:, ti:ti + 1],
                             lhsT=zsqT[:, i * P:(i + 1) * P],
                             rhs=ones_col[:], start=True, stop=True)

    class Scatter:
        """Deferred scatter/update for a finished window, advanced in
        chunks between the next window's zbuild groups so the PE FIFO
        mixes dense N=512 matmuls with the LDW-heavy scatter ops (keeps
        the HAM clock gate open)."""

        def __init__(self, w, wps, m1w, ohw):
            self.w = w
            self.wps = wps
            self.m1w = m1w
            self.ohw = ohw
            self.ntw = int(cfg.nt[w])
            self.W = self.ntw * P
            self.i = 0
            self.m2ps = []
            self.nw2 = 0

        def start(self):
            ntw, wps = self.ntw, self.wps
            std_w = stat.tile([P, max_ntw], f32, name="std_w", tag="std_w")
            nc.scalar.activation(out=std_w[:, 0:ntw], in_=wps[:, 0:ntw],
                                 func=AF.Sqrt, bias=epsD_t[:])
            self.rr_w = stat.tile([P, max_ntw], f32, name="rr_w", tag="rr_w")
            nc.vector.reciprocal(out=self.rr_w[:, 0:ntw], in_=std_w[:, 0:ntw])
            self.agg = wps[:, 32:32 + P]

        def _w2(self):
            if self.nw2 < self.ntw:
                m2p = psT.tile([P, P], f32, name="m2p", tag="m2p", bufs=3)
                nc.tensor.matmul(m2p[:], lhsT=self.m1w[:, self.nw2, :],
                                 rhs=w2s[:], start=True, stop=True)
                self.m2ps.append(m2p)
                self.nw2 += 1

        def advance(self, k):
            while k > 0 and self.i < self.ntw:
                i = self.i
                while self.nw2 < min(i + 3, self.ntw):
                    self._w2()
                m2sb = tsm.tile([P, P], bf16, name="m2sb", tag="m2sb")
                nc.vector.tensor_scalar(
                    out=m2sb[:], in0=self.m2ps[i][:],
                    scalar1=self.rr_w[:, i:i + 1],
                    scalar2=0.0, op0=OP.mult, op1=OP.max)
                nc.tensor.matmul(self.agg, lhsT=m2sb[:],
                                 rhs=self.ohw[:, i, :],
                                 start=(i == 0), stop=(i == self.ntw - 1))
                self.i += 1
                k -= 1

        def finish(self):
            self.advance(self.ntw - self.i)
            emit_update(self.w, self.wps, self.agg)

    def emit_window(w, scat):
        """Stream + zT build + m1 + sumsq for window w, interleaved with
        the previous window's scatter chunks."""
        ntw = int(cfg.nt[w])
        t0 = int(cfg.wstart[w])
        # wps bank: cols [0:max_ntw] = per-tile sumsq, cols [32:160] = agg,
        # then reused for the update-MLP matmul output.
        wps = psC.tile([P, 32 + P], f32, name="wps", tag="wps")
        m1w = wbuf.tile([P, max_ntw, P], bf16, name="m1w", tag="m1w")
        W = ntw * P
        xw_t = gin.tile([P, 2 * max_ntw * P], bf16, name="xw_t", tag="xw_t",
                        bufs=3)
        nc.sync.dma_start(out=xw_t[:, 0:2 * W],
                          in_=io["xw"][:, 2 * t0 * P:2 * t0 * P + 2 * W])
        ohw = wbuf.tile([P, max_ntw, P], bf16, name="ohw", tag="ohw")
        for i in range(ntw):
            nc.gpsimd.tensor_scalar(
                out=ohw[:, i, :], in0=iota_b[:],
                scalar1=sidxT[:, t0 + i:t0 + i + 1], scalar2=None,
                op0=OP.is_equal)
        cb_w = gin.tile([4, max_ntw * P], bf16, name="cb_w", tag="cb_w",
                        bufs=3)
        nc.sync.dma_start(out=cb_w[:, 0:W],
                          in_=io["combo4"][:, t0 * P:t0 * P + W])

        n_groups = (ntw + G - 1) // G
        chunk = ((scat.ntw + n_groups - 1) // n_groups) if scat else 0
        if scat:
            scat.start()

        pend = []
        g0 = 0
        while g0 < ntw:
            gsz = min(G, ntw - g0)
            N = gsz * P
            c0 = g0 * P

            zpT = psA.tile([P, G * P], f32, name="zpT", tag="zpT", bufs=3)
            nc.tensor.matmul(zpT[:, 0:N], lhsT=w1a[:],
                             rhs=xw_t[:, c0:c0 + N],
                             start=True, stop=False)
            nc.tensor.matmul(zpT[:, 0:N], lhsT=w1b[:],
                             rhs=xw_t[:, W + c0:W + c0 + N],
                             start=False, stop=False)
            nc.tensor.matmul(zpT[:, 0:N], lhsT=cmb[:],
                             rhs=cb_w[:, c0:c0 + N],
                             start=False, stop=True)

            zsqT = gmid.tile([P, G * P], bf16, name="zsqT", tag="zsqT",
                             bufs=3)
            nc.scalar.activation(out=zsqT[:, 0:N], in_=zpT[:, 0:N],
                                 func=AF.Square)
            if g1_col is None:
                nc.scalar.activation(out=m1w[:, g0:g0 + gsz, :],
                                     in_=zpT[:, 0:N], func=AF.Relu)
            else:
                nc.scalar.activation(out=m1w[:, g0:g0 + gsz, :],
                                     in_=zpT[:, 0:N], func=AF.Relu,
                                     scale=g1_col[:])
            pend.append((g0, gsz, zsqT))
            # interleave the previous group's sumsq columns between the big
            # zbuild matmuls so the PE array never sits in an LDW-only run
            if len(pend) > 1:
                _emit_sumsq(wps, pend.pop(0))
            if scat:
                scat.advance(chunk)
            g0 += gsz
        while pend:
            _emit_sumsq(wps, pend.pop(0))
        if scat:
            scat.finish()
        return Scatter(w, wps, m1w, ohw)

    def emit_update(w, wps, agg):
        ntw = int(cfg.nt[w])
        n0 = w * P
        # ---- update MLP
        agg_sb = wbuf.tile([P, P], bf16, name="agg_sb", tag="agg_sb")
        nc.scalar.activation(out=agg_sb[:], in_=agg, func=AF.Copy)

        up = wps[:, 32:32 + P]
        nc.tensor.matmul(up, lhsT=agg_sb[:], rhs=wua[:],
                         start=True, stop=False)
        nc.tensor.matmul(up, lhsT=xupT[:, n0:n0 + P], rhs=wux[:],
                         start=False, stop=False)
        nc.tensor.matmul(up, lhsT=ones_row[:], rhs=bu_row[:],
                         start=False, stop=True)

        st6 = stat.tile([P, 6], f32, name="st6", tag="st6")
        nc.vector.bn_stats(out=st6[:], in_=up)
        mvu = stat.tile([P, 2], f32, name="mvu", tag="mvu")
        nc.vector.bn_aggr(out=mvu[:], in_=st6[:])
        stdu = stat.tile([P, 1], f32, name="stdu", tag="stdu")
        nc.scalar.activation(out=stdu[:], in_=mvu[:, 1:2], func=AF.Sqrt,
                             bias=eps_t[:])
        rru = stat.tile([P, 1], f32, name="rru", tag="rru")
        nc.vector.reciprocal(out=rru[:], in_=stdu[:])
        nmuu = stat.tile([P, 1], f32, name="nmuu", tag="nmuu")
        nc.vector.scalar_tensor_tensor(
            out=nmuu[:], in0=mvu[:, 0:1], scalar=-1.0, in1=rru[:],
            op0=OP.mult, op1=OP.mult)

        u_sb = wbuf.tile([P, D], f32, name="u_sb", tag="u_sb")
        if triv["gu"] and triv["beu"]:
            nc.scalar.activation(out=u_sb[:], in_=up, func=AF.Relu,
                                 scale=rru[:], bias=nmuu[:])
        else:
            nc.scalar.activation(out=u_sb[:], in_=up, func=AF.Identity,
                                 scale=rru[:], bias=nmuu[:])
            nc.vector.tensor_mul(out=u_sb[:], in0=u_sb[:], in1=gu_b[:])
            nc.vector.tensor_add(out=u_sb[:], in0=u_sb[:], in1=beu_b[:])
            nc.vector.tensor_scalar_max(out=u_sb[:], in0=u_sb[:], scalar1=0.0)

        xres = wbuf.tile([P, D], f32, name="xres", tag="xres")
        nc.sync.dma_start(out=xres[:], in_=io["xres"][n0:n0 + P, :])
        o_sb = wbuf.tile([P, D], f32, name="o_sb", tag="o_sb")
        nc.vector.tensor_add(out=o_sb[:], in0=u_sb[:], in1=xres[:])
        nc.sync.dma_start(out=out_dram[n0:n0 + P, :], in_=o_sb[:])

    # PE prewarm: ~48 back-to-back matmuls release the HAM clock gate
    # (4/8 -> 8/8) before the real pipeline starts.
    warm = psT.tile([P, P], f32, name="warm", tag="m2p", bufs=3)
    for _ in range(48):
        nc.tensor.matmul(warm[:], lhsT=w2s[:], rhs=wux[:],
                         start=True, stop=True)

    # 1-window software pipeline with fine-grained interleave
    scat = None
    for w in range(cfg.n_win):
        scat = emit_window(w, scat)
    scat.start()
    scat.finish()

    ctx.close()


def make_program(cfg):
    import concourse.bacc as bacc
    import concourse.tile as tile
    from concourse import mybir

    f32 = mybir.dt.float32
    bf16 = mybir.dt.bfloat16

    nc = bacc.Bacc("TRN2", target_bir_lowering=False, debug=False,
                   num_devices=cfg.n_cores)
    io = {}

    def din(name, shape, dtype=f32):
        io[name] = nc.dram_tensor(name, list(shape), dtype,
                                  kind="ExternalInput").ap()

    din("xw", [P, 2 * cfg.e_pad], bf16)
    din("sidxT", [P, cfg.t_total])
    din("iota", [D], bf16)
    din("combo4", [4, cfg.e_pad], bf16)
    din("xupT", [P, cfg.nloc_pad], bf16)
    din("xres", [cfg.nloc_pad, D])
    din("W1a", [P, D], bf16)
    din("W1b", [P, D], bf16)
    din("combo_lhs", [4, D], bf16)
    din("W2s", [P, D], bf16)
    din("Wu_x", [P, D], bf16)
    din("Wu_a", [P, D], bf16)
    din("bu_row", [1, D], bf16)
    if not cfg.triv["g1"]:
        din("g1_col", [P, 1])
    if not cfg.triv["gu"]:
        din("gu", [D])
    if not cfg.triv["beu"]:
        din("beu", [D])
    io["out"] = nc.dram_tensor("out", [cfg.nloc_pad, D], f32,
                               kind="ExternalOutput").ap()

    with tile.TileContext(nc) as tc:
        build(nc, tc, cfg, io)
    nc.compile()
    return nc


# ---------------------------------------------------------------- entry

def _is_const(v, val):
    return bool(np.allclose(np.asarray(v), val))


def kernel(x, edge_index, edge_attr, W1, b1, g1, be1, W2, b2, Wu, bu, gu, beu,
           cfg=None, run=True):
    x = np.ascontiguousarray(np.asarray(x, dtype=np.float32))
    edge_index = np.asarray(edge_index)
    edge_attr = np.asarray(edge_attr, dtype=np.float32)
    W1 = np.asarray(W1, dtype=np.float32)
    b1 = np.asarray(b1, dtype=np.float32)
    W2 = np.asarray(W2, dtype=np.float32)
    Wu = np.asarray(Wu, dtype=np.float32)

    if not _is_const(be1, 0) or not _is_const(b2, 0):
        raise NotImplementedError("be1/b2 must be zero for this kernel")

    if cfg is None:
        cfg = Cfg(N_NODES, N_EDGES, N_CORES)
    cfg.triv = dict(
        g1=_is_const(g1, 1), gu=_is_const(gu, 1), beu=_is_const(beu, 0),
    )

    w1sum = W1.sum(axis=1)
    b1sum = float(np.sum(b1))
    core_arrays = preprocess(cfg, x, edge_index, edge_attr, w1sum, b1sum)

    weights = dict(
        W1a=W1[0:D].astype(BF16),
        W1b=W1[D:2 * D].astype(BF16),
        combo_lhs=np.stack(
            [W1[2 * D], W1[2 * D + 1], np.ones(D, np.float32), b1]
        ).astype(BF16),
        W2s=(W2 * math.sqrt(D)).astype(BF16),
        Wu_x=Wu[0:D].astype(BF16),
        Wu_a=Wu[D:2 * D].astype(BF16),
        bu_row=np.asarray(bu, np.float32).reshape(1, D).astype(BF16),
        iota=np.arange(D, dtype=np.float32).astype(BF16),
    )
    if not cfg.triv["g1"]:
        weights["g1_col"] = np.asarray(g1, np.float32).reshape(P, 1)
    if not cfg.triv["gu"]:
        weights["gu"] = np.asarray(gu, np.float32)
    if not cfg.triv["beu"]:
        weights["beu"] = np.asarray(beu, np.float32)

    nc = make_program(cfg)

    in_maps = []
    for k in range(cfg.n_cores):
        xk = x[k * cfg.nloc:(k + 1) * cfg.nloc]
        perm = cfg.perms[k].reshape(-1)
        valid = perm >= 0
        xl = np.zeros((cfg.nloc_pad, D), dtype=np.float32)
        xl[valid] = xk[perm[valid]]
        m = dict(core_arrays[k])
        m["xres"] = xl
        m["xupT"] = np.ascontiguousarray(xl.T.astype(BF16))
        m.update(weights)
        in_maps.append(m)

    if not run:
        return nc, in_maps, cfg

    from concourse import bass_utils
    res = bass_utils.run_bass_kernel_spmd(
        nc, in_maps, core_ids=list(range(cfg.n_cores)),
        trace=bool(int(os.environ.get("KERNEL_TRACE", "0"))),
    )
    kernel.last_results = res
    outs = []
    for k, r in enumerate(res.results):
        perm = cfg.perms[k].reshape(-1)
        valid = perm >= 0
        ok = np.empty((cfg.nloc, D), dtype=np.float32)
        ok[perm[valid]] = r["out"][valid]
        outs.append(ok)
    return np.concatenate(outs, axis=0)


kernel.last_results = None


# revision 27
# speedup vs baseline: 3.7269x; 3.7269x over previous
"""Trainium2 Bass kernel for a GNN message-passing layer (8-core SPMD), v2.

Math (reference):
    z   = [x[row], x[col], edge_attr] @ W1 + b1        # [E, 258] @ [258, 128]
    m   = relu(LN(z, g1, be1))
    m   = relu(m @ W2 + b2)
    aggr= segment_sum(m, row, N)
    u   = relu(LN([x, aggr] @ Wu + bu, gu, beu))
    out = x + u

Design (v2 — no device gathers at all):
  The host shards edges by source-row range (nloc rows per core), sorts each
  shard by 128-row window, and pre-gathers x[row], x[col] per edge into
  transposed bf16 tile streams.  The device streams those sequentially
  (HWDGE DMA) and computes, per 128-edge tile group (4 tiles = N=512):

    zT [D, e] = W1a^T @ xrowT + W1b^T @ xcolT + [c0;c1;1;b1]^T @ [a0;a1;-mu;v]
      (mean subtracted inside the PSUM accumulation: -mu is a host-computed
       per-edge scalar = -(h.w1sum + sum b1)/D, a linear functional of the
       inputs; v is the valid mask so pad edges stay exactly zero)
    m1u = relu(g1 * zT)            # ACT batched, rstd deferred (exact: rstd>0)
    m2T = relu(W2s^T @ m1u)        # W2s = W2 * sqrt(D), rstd deferred again
    zsq = Square(zT)               # ACT batched -> DMA-transpose -> DVE
    sumsq[e] = reduce(zsq)         #   row-sum; var = sumsq/D (mean folded)
    rr = 1/sqrt(sumsq + D*eps)     # per window: ACT Sqrt + DVE reciprocal
    oh = (iota == sidx) * rr       # one DVE tensor_scalar, bf16
    aggT [D, slot] += m2^T @ oh    # PE scatter; m2 via DMA-transpose of m2T

  Update MLP per 128-node window: up = aggT^T@Wu_a + x@Wu_x + bu, LN+relu on
  ACT/DVE, residual add, DMA out.  No collectives, no GPSIMD, no dma_gather.

  One NEFF for all 8 cores: per-window tile counts are maxed across cores;
  pad edges are all-zero with sidx=-1 (one-hot column is all zero).
"""

import math
import os
import numpy as np
import ml_dtypes

BF16 = ml_dtypes.bfloat16

D = 128
N_NODES = 50000
N_EDGES = 800000
N_CORES = 8
EPS = 1e-5
P = 128
G = 4  # tiles per matmul group (N = G*128 = 512 <= one PSUM bank of fp32)


class Cfg:
    def __init__(self, n_nodes, n_edges, n_cores):
        self.n_nodes = n_nodes
        self.n_edges = n_edges
        self.n_cores = n_cores
        assert n_nodes % n_cores == 0
        self.nloc = n_nodes // n_cores
        self.n_win = math.ceil(self.nloc / P)
        self.nloc_pad = self.n_win * P
        self.nt = None        # [n_win] tiles per window (maxed over cores)
        self.t_total = None
        self.e_pad = None
        self.max_ntw = None
        self.wstart = None    # [n_win] first tile index of each window
        self.perms = None     # per-core [n_win, 128] node permutation
        self.triv = None


# ---------------------------------------------------------------- host prep

def _pack_windows(deg, n_win, cap):
    """First/best-fit-decreasing: assign nodes to n_win bins of <=128 nodes,
    targeting <=cap edges per bin.  Returns win_of[node], bin edge counts."""
    nloc = len(deg)
    order = np.argsort(-deg, kind="stable")
    bins_e = np.zeros(n_win, dtype=np.int64)
    bins_n = np.zeros(n_win, dtype=np.int64)
    win_of = np.full(nloc, -1, dtype=np.int64)
    for n in order:
        d = int(deg[n])
        ok = (bins_e + d <= cap) & (bins_n < P)
        if ok.any():
            cand = np.where(ok)[0]
            b = cand[np.argmax(bins_e[cand])]          # fullest that fits
        else:
            cand = np.where(bins_n < P)[0]
            b = cand[np.argmin(bins_e[cand])]          # least-loaded spill
        win_of[n] = b
        bins_e[b] += d
        bins_n[b] += 1
    return win_of, bins_e


def preprocess(cfg, x, edge_index, edge_attr, w1sum, b1sum):
    """Shard edges, bin-pack nodes into 128-node windows with balanced edge
    counts, pre-gather x rows, build per-core bf16 streams."""
    rows = np.asarray(edge_index[0], dtype=np.int64)
    cols = np.asarray(edge_index[1], dtype=np.int64)
    attr = np.asarray(edge_attr, dtype=np.float32)

    # per-node linear functionals for the LN mean
    xw1a = x @ w1sum[0:D]          # [N]
    xw1b = x @ w1sum[D:2 * D]      # [N]

    order = np.argsort(rows, kind="stable")
    rs = rows[order]
    bounds = np.searchsorted(rs, np.arange(cfg.n_cores + 1) * cfg.nloc)

    per_core = []
    counts = np.zeros((cfg.n_cores, cfg.n_win), dtype=np.int64)
    perms = []
    for k in range(cfg.n_cores):
        sel = order[bounds[k]:bounds[k + 1]]
        row = rows[sel]
        row_l = (row - k * cfg.nloc).astype(np.int64)
        col = cols[sel]
        at = attr[sel]

        deg = np.bincount(row_l, minlength=cfg.nloc)
        cap = P * max(1, int(math.ceil(len(row_l) / (cfg.n_win * P))))
        win_of, bins_e = _pack_windows(deg, cfg.n_win, cap)
        rank = np.argsort(-bins_e, kind="stable")     # big windows first
        rankpos = np.empty(cfg.n_win, dtype=np.int64)
        rankpos[rank] = np.arange(cfg.n_win)
        wpos_of = rankpos[win_of]                     # node -> window index
        # slots within each window, and the node permutation
        o_n = np.argsort(wpos_of, kind="stable")
        slot_of = np.empty(cfg.nloc, dtype=np.int64)
        perm = np.full((cfg.n_win, P), -1, dtype=np.int64)
        wcnt = np.bincount(wpos_of, minlength=cfg.n_win)
        off = 0
        for w in range(cfg.n_win):
            nn = int(wcnt[w])
            nodes = o_n[off:off + nn]
            slot_of[nodes] = np.arange(nn)
            perm[w, :nn] = nodes
            off += nn
        perms.append(perm)

        key = wpos_of[row_l]
        o2 = np.argsort(key, kind="stable")
        row_l, col, at, row = row_l[o2], col[o2], at[o2], row[o2]
        counts[k] = np.bincount(key[o2], minlength=cfg.n_win)
        per_core.append((slot_of[row_l], col, at, row))

    cfg.perms = perms
    nt = np.maximum(np.ceil(counts.max(axis=0) / P).astype(np.int64), 1)
    cfg.nt = nt
    cfg.t_total = int(nt.sum())
    cfg.e_pad = cfg.t_total * P
    cfg.max_ntw = int(nt.max())
    cfg.wstart = np.concatenate([[0], np.cumsum(nt)[:-1]]).astype(np.int64)

    x_bf = x.astype(BF16)
    core_arrays = []
    for k in range(cfg.n_cores):
        slot, col, at, row = per_core[k]
        # destination position of each (sorted) edge in the padded stream
        pos = np.empty(len(slot), dtype=np.int64)
        src = 0
        for w in range(cfg.n_win):
            c = int(counts[k, w])
            pos[src:src + c] = cfg.wstart[w] * P + np.arange(c)
            src += c

        xrow = np.zeros((cfg.e_pad, D), dtype=BF16)
        xcol = np.zeros((cfg.e_pad, D), dtype=BF16)
        combo = np.zeros((4, cfg.e_pad), dtype=np.float32)
        sidx = np.full(cfg.e_pad, -1.0, dtype=np.float32)

        xrow[pos] = x_bf[row]
        xcol[pos] = x_bf[col]
        combo[0, pos] = at[:, 0]
        combo[1, pos] = at[:, 1]
        negmu = -(xw1a[row] + xw1b[col]
                  + at[:, 0] * w1sum[2 * D] + at[:, 1] * w1sum[2 * D + 1]
                  + b1sum) / D
        combo[2, pos] = negmu
        combo[3, pos] = 1.0
        sidx[pos] = slot.astype(np.float32)

        # per-window [xrowT_w | xcolT_w | onehot_w] stream: one DMA/window.
        # The scatter one-hot is static given the edge list, so the host
        # builds it (plain 0/1 bf16); rstd is applied on the m2 side.
        xrowT = xrow.T
        xcolT = xcol.T
        xw = np.zeros((P, 3 * cfg.e_pad), dtype=BF16)
        for w in range(cfg.n_win):
            W = int(cfg.nt[w]) * P
            t0c = int(cfg.wstart[w]) * P
            b = 3 * t0c
            xw[:, b:b + W] = xrowT[:, t0c:t0c + W]
            xw[:, b + W:b + 2 * W] = xcolT[:, t0c:t0c + W]
        # one-hot: edge at padded position p (tile ti, lane p%128) sets
        # block col 2*W + (ti-wstart)*128 + sidx
        tile_w = np.repeat(np.arange(cfg.n_win), cfg.nt)      # [t_total]
        Wcols = (cfg.nt * P)[tile_w]
        tile_col0 = (3 * cfg.wstart[tile_w] * P + 2 * Wcols
                     + (np.arange(cfg.t_total) - cfg.wstart[tile_w]) * P)
        ti_e = pos // P
        lane_e = pos % P
        col_e = tile_col0[ti_e] + sidx[pos].astype(np.int64)
        xw[lane_e, col_e] = 1.0
        core_arrays.append(dict(
            xw=xw,
            combo4=combo.astype(BF16),
        ))
    return core_arrays


# ---------------------------------------------------------------- device IR

def build(nc, tc, cfg, io):
    from concourse import mybir
    from contextlib import ExitStack
    import concourse.bass as bass

    f32 = mybir.dt.float32
    bf16 = mybir.dt.bfloat16
    AF = mybir.ActivationFunctionType
    OP = mybir.AluOpType
    AX = mybir.AxisListType
    triv = cfg.triv

    ctx = ExitStack()
    sing = ctx.enter_context(tc.tile_pool(name="sing", bufs=1))
    gin = ctx.enter_context(tc.tile_pool(name="gin", bufs=3))
    gmid = ctx.enter_context(tc.tile_pool(name="gmid", bufs=2))
    tsm = ctx.enter_context(tc.tile_pool(name="tsm", bufs=3))
    wbuf = ctx.enter_context(tc.tile_pool(name="wbuf", bufs=2))
    stat = ctx.enter_context(tc.tile_pool(name="stat", bufs=2))
    psA = ctx.enter_context(tc.tile_pool(name="psA", bufs=3, space="PSUM"))
    psC = ctx.enter_context(tc.tile_pool(name="psC", bufs=2, space="PSUM"))
    psT = ctx.enter_context(tc.tile_pool(name="psT", bufs=3, space="PSUM"))

    # ---------------- resident weights / constants
    def load(name, shape, dt):
        t = sing.tile(shape, dt, name=f"{name}_sb")
        nc.sync.dma_start(out=t[:], in_=io[name][:])
        return t

    w1a = load("W1a", [P, D], bf16)
    w1b = load("W1b", [P, D], bf16)
    cmb = load("combo_lhs", [4, D], bf16)
    w2s = load("W2s", [P, D], bf16)
    wux = load("Wu_x", [P, D], bf16)
    wua = load("Wu_a", [P, D], bf16)
    bu_row = load("bu_row", [1, D], bf16)
    xupT = load("xupT", [P, cfg.nloc_pad], bf16)

    def bcast_row(name, dt):
        t = sing.tile([P, D], dt, name=f"{name}_b")
        src = io[name]
        ap = bass.AP(tensor=src.tensor, offset=src.offset,
                     ap=[[0, P]] + list(src.ap))
        nc.sync.dma_start(out=t[:], in_=ap)
        return t

    g1_col = None if triv["g1"] else load("g1_col", [P, 1], f32)
    gu_b = None if triv["gu"] else bcast_row("gu", f32)
    beu_b = None if triv["beu"] else bcast_row("beu", f32)

    ones_row = sing.tile([1, P], bf16, name="ones_row")
    nc.vector.memset(ones_row[:], 1.0)
    ones_col = sing.tile([P, 1], bf16, name="ones_col")
    nc.vector.memset(ones_col[:], 1.0)
    epsD_t = sing.tile([P, 1], f32, name="epsD_t")
    nc.vector.memset(epsD_t[:], float(D * EPS))
    eps_t = sing.tile([P, 1], f32, name="eps_t")
    nc.vector.memset(eps_t[:], EPS)

    from concourse.masks import make_identity
    ident_bf = sing.tile([P, P], bf16, name="ident_bf")
    make_identity(nc, ident_bf[:])

    out_dram = io["out"]
    max_ntw = cfg.max_ntw

    # per-window state produced by the group phase, consumed by scatter
    def _emit_sumsq(wps, ent):
        gg, gsz, zsqT = ent
        for i in range(gsz):
            ti = gg + i
            nc.tensor.matmul(wps[:, ti:ti + 1],
                             lhsT=zsqT[:, i * P:(i + 1) * P],
                             rhs=ones_col[:], start=True, stop=True)

    class Scatter:
        """Deferred scatter/update for a finished window, advanced in
        chunks between the next window's zbuild groups so the PE FIFO
        mixes dense N=512 matmuls with the LDW-heavy scatter ops (keeps
        the HAM clock gate open)."""

        def __init__(self, w, wps, m1w, xw_t):
            self.w = w
            self.wps = wps
            self.m1w = m1w
            self.xw_t = xw_t
            self.ntw = int(cfg.nt[w])
            self.W = self.ntw * P
            self.i = 0
            self.m2ps = []
            self.nw2 = 0

        def start(self):
            ntw, wps = self.ntw, self.wps
            std_w = stat.tile([P, max_ntw], f32, name="std_w", tag="std_w")
            nc.scalar.activation(out=std_w[:, 0:ntw], in_=wps[:, 0:ntw],
                                 func=AF.Sqrt, bias=epsD_t[:])
            self.rr_w = stat.tile([P, max_ntw], f32, name="rr_w", tag="rr_w")
            nc.vector.reciprocal(out=self.rr_w[:, 0:ntw], in_=std_w[:, 0:ntw])
            self.agg = wps[:, 32:32 + P]

        def _w2(self):
            if self.nw2 < self.ntw:
                m2p = psT.tile([P, P], f32, name="m2p", tag="m2p", bufs=3)
                nc.tensor.matmul(m2p[:], lhsT=self.m1w[:, self.nw2, :],
                                 rhs=w2s[:], start=True, stop=True)
                self.m2ps.append(m2p)
                self.nw2 += 1

        def advance(self, k):
            while k > 0 and self.i < self.ntw:
                i = self.i
                while self.nw2 < min(i + 3, self.ntw):
                    self._w2()
                m2sb = tsm.tile([P, P], bf16, name="m2sb", tag="m2sb")
                nc.vector.tensor_scalar(
                    out=m2sb[:], in0=self.m2ps[i][:],
                    scalar1=self.rr_w[:, i:i + 1],
                    scalar2=0.0, op0=OP.mult, op1=OP.max)
                oh = self.xw_t[:, 2 * self.W + i * P:2 * self.W + (i + 1) * P]
                nc.tensor.matmul(self.agg, lhsT=m2sb[:], rhs=oh,
                                 start=(i == 0), stop=(i == self.ntw - 1))
                self.i += 1
                k -= 1

        def finish(self):
            self.advance(self.ntw - self.i)
            emit_update(self.w, self.wps, self.agg)

    def emit_window(w, scat):
        """Stream + zT build + m1 + sumsq for window w, interleaved with
        the previous window's scatter chunks."""
        ntw = int(cfg.nt[w])
        t0 = int(cfg.wstart[w])
        # wps bank: cols [0:max_ntw] = per-tile sumsq, cols [32:160] = agg,
        # then reused for the update-MLP matmul output.
        wps = psC.tile([P, 32 + P], f32, name="wps", tag="wps")
        m1w = wbuf.tile([P, max_ntw, P], bf16, name="m1w", tag="m1w")
        W = ntw * P
        xw_t = gin.tile([P, 3 * max_ntw * P], bf16, name="xw_t", tag="xw_t",
                        bufs=3)
        nc.sync.dma_start(out=xw_t[:, 0:3 * W],
                          in_=io["xw"][:, 3 * t0 * P:3 * t0 * P + 3 * W])
        cb_w = gin.tile([4, max_ntw * P], bf16, name="cb_w", tag="cb_w",
                        bufs=3)
        nc.sync.dma_start(out=cb_w[:, 0:W],
                          in_=io["combo4"][:, t0 * P:t0 * P + W])

        n_groups = (ntw + G - 1) // G
        chunk = ((scat.ntw + n_groups - 1) // n_groups) if scat else 0
        if scat:
            scat.start()

        pend = []
        g0 = 0
        while g0 < ntw:
            gsz = min(G, ntw - g0)
            N = gsz * P
            c0 = g0 * P

            zpT = psA.tile([P, G * P], f32, name="zpT", tag="zpT", bufs=3)
            nc.tensor.matmul(zpT[:, 0:N], lhsT=w1a[:],
                             rhs=xw_t[:, c0:c0 + N],
                             start=True, stop=False)
            nc.tensor.matmul(zpT[:, 0:N], lhsT=w1b[:],
                             rhs=xw_t[:, W + c0:W + c0 + N],
                             start=False, stop=False)
            nc.tensor.matmul(zpT[:, 0:N], lhsT=cmb[:],
                             rhs=cb_w[:, c0:c0 + N],
                             start=False, stop=True)

            zsqT = gmid.tile([P, G * P], bf16, name="zsqT", tag="zsqT",
                             bufs=3)
            nc.scalar.activation(out=zsqT[:, 0:N], in_=zpT[:, 0:N],
                                 func=AF.Square)
            if g1_col is None:
                nc.scalar.activation(out=m1w[:, g0:g0 + gsz, :],
                                     in_=zpT[:, 0:N], func=AF.Relu)
            else:
                nc.scalar.activation(out=m1w[:, g0:g0 + gsz, :],
                                     in_=zpT[:, 0:N], func=AF.Relu,
                                     scale=g1_col[:])
            pend.append((g0, gsz, zsqT))
            # interleave the previous group's sumsq columns between the big
            # zbuild matmuls so the PE array never sits in an LDW-only run
            if len(pend) > 1:
                _emit_sumsq(wps, pend.pop(0))
            if scat:
                scat.advance(chunk)
            g0 += gsz
        while pend:
            _emit_sumsq(wps, pend.pop(0))
        if scat:
            scat.finish()
        return Scatter(w, wps, m1w, xw_t)

    def emit_update(w, wps, agg):
        ntw = int(cfg.nt[w])
        n0 = w * P
        # ---- update MLP
        agg_sb = wbuf.tile([P, P], bf16, name="agg_sb", tag="agg_sb")
        nc.scalar.activation(out=agg_sb[:], in_=agg, func=AF.Copy)

        up = wps[:, 32:32 + P]
        nc.tensor.matmul(up, lhsT=agg_sb[:], rhs=wua[:],
                         start=True, stop=False)
        nc.tensor.matmul(up, lhsT=xupT[:, n0:n0 + P], rhs=wux[:],
                         start=False, stop=False)
        nc.tensor.matmul(up, lhsT=ones_row[:], rhs=bu_row[:],
                         start=False, stop=True)

        st6 = stat.tile([P, 6], f32, name="st6", tag="st6")
        nc.vector.bn_stats(out=st6[:], in_=up)
        mvu = stat.tile([P, 2], f32, name="mvu", tag="mvu")
        nc.vector.bn_aggr(out=mvu[:], in_=st6[:])
        stdu = stat.tile([P, 1], f32, name="stdu", tag="stdu")
        nc.scalar.activation(out=stdu[:], in_=mvu[:, 1:2], func=AF.Sqrt,
                             bias=eps_t[:])
        rru = stat.tile([P, 1], f32, name="rru", tag="rru")
        nc.vector.reciprocal(out=rru[:], in_=stdu[:])
        nmuu = stat.tile([P, 1], f32, name="nmuu", tag="nmuu")
        nc.vector.scalar_tensor_tensor(
            out=nmuu[:], in0=mvu[:, 0:1], scalar=-1.0, in1=rru[:],
            op0=OP.mult, op1=OP.mult)

        u_sb = wbuf.tile([P, D], f32, name="u_sb", tag="u_sb")
        if triv["gu"] and triv["beu"]:
            nc.scalar.activation(out=u_sb[:], in_=up, func=AF.Relu,
                                 scale=rru[:], bias=nmuu[:])
        else:
            nc.scalar.activation(out=u_sb[:], in_=up, func=AF.Identity,
                                 scale=rru[:], bias=nmuu[:])
            nc.vector.tensor_mul(out=u_sb[:], in0=u_sb[:], in1=gu_b[:])
            nc.vector.tensor_add(out=u_sb[:], in0=u_sb[:], in1=beu_b[:])
            nc.vector.tensor_scalar_max(out=u_sb[:], in0=u_sb[:], scalar1=0.0)

        xres = wbuf.tile([P, D], f32, name="xres", tag="xres")
        nc.sync.dma_start(out=xres[:], in_=io["xres"][n0:n0 + P, :])
        o_sb = wbuf.tile([P, D], f32, name="o_sb", tag="o_sb")
        nc.vector.tensor_add(out=o_sb[:], in0=u_sb[:], in1=xres[:])
        nc.sync.dma_start(out=out_dram[n0:n0 + P, :], in_=o_sb[:])

    # PE prewarm: ~48 back-to-back matmuls release the HAM clock gate
    # (4/8 -> 8/8) before the real pipeline starts.
    warm = psT.tile([P, P], f32, name="warm", tag="m2p", bufs=3)
    for _ in range(48):
        nc.tensor.matmul(warm[:], lhsT=w2s[:], rhs=wux[:],
                         start=True, stop=True)

    # 1-window software pipeline with fine-grained interleave
    scat = None
    for w in range(cfg.n_win):
        scat = emit_window(w, scat)
    scat.start()
    scat.finish()

    ctx.close()


def make_program(cfg):
    import concourse.bacc as bacc
    import concourse.tile as tile
    from concourse import mybir

    f32 = mybir.dt.float32
    bf16 = mybir.dt.bfloat16

    nc = bacc.Bacc("TRN2", target_bir_lowering=False, debug=False,
                   num_devices=cfg.n_cores)
    io = {}

    def din(name, shape, dtype=f32):
        io[name] = nc.dram_tensor(name, list(shape), dtype,
                                  kind="ExternalInput").ap()

    din("xw", [P, 3 * cfg.e_pad], bf16)
    din("combo4", [4, cfg.e_pad], bf16)
    din("xupT", [P, cfg.nloc_pad], bf16)
    din("xres", [cfg.nloc_pad, D])
    din("W1a", [P, D], bf16)
    din("W1b", [P, D], bf16)
    din("combo_lhs", [4, D], bf16)
    din("W2s", [P, D], bf16)
    din("Wu_x", [P, D], bf16)
    din("Wu_a", [P, D], bf16)
    din("bu_row", [1, D], bf16)
    if not cfg.triv["g1"]:
        din("g1_col", [P, 1])
    if not cfg.triv["gu"]:
        din("gu", [D])
    if not cfg.triv["beu"]:
        din("beu", [D])
    io["out"] = nc.dram_tensor("out", [cfg.nloc_pad, D], f32,
                               kind="ExternalOutput").ap()

    with tile.TileContext(nc) as tc:
        build(nc, tc, cfg, io)
    nc.compile()
    return nc


# ---------------------------------------------------------------- entry

def _is_const(v, val):
    return bool(np.allclose(np.asarray(v), val))


def kernel(x, edge_index, edge_attr, W1, b1, g1, be1, W2, b2, Wu, bu, gu, beu,
           cfg=None, run=True):
    x = np.ascontiguousarray(np.asarray(x, dtype=np.float32))
    edge_index = np.asarray(edge_index)
    edge_attr = np.asarray(edge_attr, dtype=np.float32)
    W1 = np.asarray(W1, dtype=np.float32)
    b1 = np.asarray(b1, dtype=np.float32)
    W2 = np.asarray(W2, dtype=np.float32)
    Wu = np.asarray(Wu, dtype=np.float32)

    if not _is_const(be1, 0) or not _is_const(b2, 0):
        raise NotImplementedError("be1/b2 must be zero for this kernel")

    if cfg is None:
        cfg = Cfg(N_NODES, N_EDGES, N_CORES)
    cfg.triv = dict(
        g1=_is_const(g1, 1), gu=_is_const(gu, 1), beu=_is_const(beu, 0),
    )

    w1sum = W1.sum(axis=1)
    b1sum = float(np.sum(b1))
    core_arrays = preprocess(cfg, x, edge_index, edge_attr, w1sum, b1sum)

    weights = dict(
        W1a=W1[0:D].astype(BF16),
        W1b=W1[D:2 * D].astype(BF16),
        combo_lhs=np.stack(
            [W1[2 * D], W1[2 * D + 1], np.ones(D, np.float32), b1]
        ).astype(BF16),
        W2s=(W2 * math.sqrt(D)).astype(BF16),
        Wu_x=Wu[0:D].astype(BF16),
        Wu_a=Wu[D:2 * D].astype(BF16),
        bu_row=np.asarray(bu, np.float32).reshape(1, D).astype(BF16),
    )
    if not cfg.triv["g1"]:
        weights["g1_col"] = np.asarray(g1, np.float32).reshape(P, 1)
    if not cfg.triv["gu"]:
        weights["gu"] = np.asarray(gu, np.float32)
    if not cfg.triv["beu"]:
        weights["beu"] = np.asarray(beu, np.float32)

    nc = make_program(cfg)

    in_maps = []
    for k in range(cfg.n_cores):
        xk = x[k * cfg.nloc:(k + 1) * cfg.nloc]
        perm = cfg.perms[k].reshape(-1)
        valid = perm >= 0
        xl = np.zeros((cfg.nloc_pad, D), dtype=np.float32)
        xl[valid] = xk[perm[valid]]
        m = dict(core_arrays[k])
        m["xres"] = xl
        m["xupT"] = np.ascontiguousarray(xl.T.astype(BF16))
        m.update(weights)
        in_maps.append(m)

    if not run:
        return nc, in_maps, cfg

    from concourse import bass_utils
    res = bass_utils.run_bass_kernel_spmd(
        nc, in_maps, core_ids=list(range(cfg.n_cores)),
        trace=bool(int(os.environ.get("KERNEL_TRACE", "0"))),
    )
    kernel.last_results = res
    outs = []
    for k, r in enumerate(res.results):
        perm = cfg.perms[k].reshape(-1)
        valid = perm >= 0
        ok = np.empty((cfg.nloc, D), dtype=np.float32)
        ok[perm[valid]] = r["out"][valid]
        outs.append(ok)
    return np.concatenate(outs, axis=0)


kernel.last_results = None
